# revision 2
# baseline (speedup 1.0000x reference)
"""Trainium2 Bass kernel for nn_CANNLinear (4-bit per-tensor symmetric weight
quantization + dense linear), column-parallel over 8 NeuronCores.

Computation (matches the reference exactly at the quantization step):
    scale  = max(max(|W|) * fl(1/7), 1e-8)        (global over full W, AllReduce max)
    q      = round(W * fl(1/scale))               (RNE round via +/-1.5*2^23)
    out    = x @ (q * scale)^T + bias

Matmul runs in mixed precision to hit the fp8 double-pump (157 TF/s) while
staying inside the 2e-2 rel-err gate:
  - KTA k-tiles: x cast f32->bf16->e4m3, q cast to e4m3 (exact: q in [-8,7]),
    DoubleRow perf mode consumes k-tile PAIRS at 2x bf16 rate.
  - KTB tail k-tiles: x bf16, q bf16 (exact), normal matmul.
Host-side exact simulation on the benchmark data (sim_err.py): KTB=20 ->
rel err 0.0167; pure fp8 would be 0.0283 (fail), pure bf16 0.0017.

Sharding: W/bias split along OUT across 8 cores (column parallel), x replicated,
per-core output [N, OUT/8] concatenated on the host along axis 1.

Per-core program:
  1. bias broadcast to [128, OUT_SH] via PE outer product (off critical path)
  2. absmax: stream W, DVE abs-max reduce -> AllReduce(max) -> scale, 1/scale
  3. quantize: re-stream W, q_bf16 = (w*inv + M) - M -> q_dram (8 panels of
     512 cols), xbar DMA-transpose panels -> fp8 cast (first KTA tiles) or
     direct bf16 residency (tail KTB tiles); low-k matmuls start while later
     panels still stream
  4. main: per 128-row block: load x halves, ACT cast bf16, xbar-transpose,
     DVE cast tiles 0..KTA-1 to e4m3; 4 PSUM banks accumulate KTA/2 DoubleRow
     + KTB bf16 matmuls; epilogue ACT copy*scale + DVE bias add -> DMA out
"""

import numpy as np

import concourse.bass as bass
import concourse.mybir as mybir
import concourse.tile as tile
from concourse import bacc
from concourse.bass_utils import run_bass_kernel_spmd

f32 = mybir.dt.float32
bf16 = mybir.dt.bfloat16
f8e4 = mybir.dt.float8e4
FP_MAGIC = 12582912.0  # 1.5 * 2**23: v + FP_MAGIC - FP_MAGIC == round-half-even(v)
QMAX = 7.0
R7 = float(np.float32(1.0) / np.float32(7.0))  # fl(1/7)
EPS = 1e-8

N_FULL, IN_FULL, OUT_FULL = 8192, 4096, 16384
CORES = 8
KTB = 20  # bf16 tail k-tiles of kt=32; KTA = kt - KTB fp8 (must be even)


def declare_io(nc, n, in_, out_sh):
    xd = nc.dram_tensor("x", [n, in_], f32, kind="ExternalInput").ap()
    wd = nc.dram_tensor("weight", [out_sh, in_], f32, kind="ExternalInput").ap()
    bd = nc.dram_tensor("bias", [out_sh], f32, kind="ExternalInput").ap()
    outd = nc.dram_tensor("out", [n, out_sh], f32, kind="ExternalOutput").ap()
    return xd, wd, bd, outd


_REP_ID = [0]


def emit_program(tc, n, in_, out_sh, n_cores, io=None):
    nc = tc.nc
    if io is None:
        io = declare_io(nc, n, in_, out_sh)
    xd, wd, bd, outd = io
    rid = _REP_ID[0]
    _REP_ID[0] += 1
    add = mybir.AluOpType.add
    sub = mybir.AluOpType.subtract
    mult = mybir.AluOpType.mult
    mx = mybir.AluOpType.max
    ax_x = mybir.AxisListType.X
    drow = mybir.MatmulPerfMode.DoubleRow

    kt = in_ // 128          # contraction tiles (32)
    kta = kt - KTB           # fp8 double-row tiles
    assert kta % 2 == 0 and kta >= 0
    nb = n // 128            # row blocks
    ot = min(512, out_sh)    # psum tile free dim
    not_ = out_sh // ot      # psum groups per row block (<=4 for 8 banks)
    assert not_ <= 4
    wrows = out_sh // 128
    wc = min(in_, 2048)      # quant load chunk columns
    nwc = in_ // wc
    tch = 512                # transpose panel columns (4 k-tiles)
    nch = in_ // tch
    tpc = tch // 128         # k-tiles per panel (4)
    assert kta % tpc == 0, "fp8/bf16 boundary must align to transpose panels"
    xc = min(in_, 2048)      # x load chunk columns
    nxc = in_ // xc
    xkc = xc // 128

    qds = [nc.dram_tensor(f"q_dram{rid}_{c}", [out_sh, tch], bf16).ap()
           for c in range(nch)]
    cc_in = nc.dram_tensor(f"cc_in{rid}", [1], f32).ap()
    cc_out = nc.dram_tensor(f"cc_out{rid}", [1], f32, addr_space="Shared").ap()

    from contextlib import ExitStack

    with ExitStack() as ctx:
        const = ctx.enter_context(tc.tile_pool(name="const", bufs=1))
        xfp = ctx.enter_context(tc.tile_pool(name="xf", bufs=2))
        xbp = ctx.enter_context(tc.tile_pool(name="xb", bufs=2))
        xtp = ctx.enter_context(tc.tile_pool(name="xt", bufs=2))
        x8p = ctx.enter_context(tc.tile_pool(name="x8", bufs=2))
        obp = ctx.enter_context(tc.tile_pool(name="ob", bufs=2))
        wtp = ctx.enter_context(tc.tile_pool(name="wt", bufs=1))

        # one padded slot holds all tiny scalar tiles
        nwt = wrows * (in_ // min(in_, 1024))
        misc = const.tile([128, 272 + nwt], f32, tag="misc")
        ones = misc[0:1, 0:128]
        scale_col = misc[:, 256:257]
        inv_col = misc[:, 257:258]
        amax = misc[0:1, 259:260]
        scale_s = misc[0:1, 260:261]
        part = misc[:, 272:272 + nwt]
        bias_rep = const.tile([128, out_sh], f32, tag="bias_rep")
        wt8 = (wtp.tile([128, kta, out_sh], f8e4, tag="wt8") if kta else None)
        wtb = (wtp.tile([128, KTB, out_sh], bf16, tag="wtb") if KTB else None)

        nc.vector.memset(ones, 1.0)

        # ---- bias broadcast (independent of everything else) ----
        with tc.tile_pool(name="psprep", bufs=2, space="PSUM") as psprep:
            nc.sync.dma_start(bias_rep[0:1, :], bd)
            for j in range(out_sh // ot):
                pbias = psprep.tile([128, ot], f32, tag="brd", name="pbias")
                nc.tensor.matmul(pbias[:], ones,
                                 bias_rep[0:1, j * ot:(j + 1) * ot],
                                 start=True, stop=True)
                nc.scalar.copy(bias_rep[:, j * ot:(j + 1) * ot], pbias[:])

            # ---- absmax -> scale (own deep pool: DMA-rate streaming) ----
            awc = min(in_, 1024)
            anwc = in_ // awc
            with tc.tile_pool(name="wabs", bufs=6) as wabs:
                for t in range(wrows):
                    for c in range(anwc):
                        wt_ = wabs.tile([128, awc], f32, tag="aload")
                        nc.sync.dma_start(wt_[:], wd[t * 128:(t + 1) * 128,
                                                     c * awc:(c + 1) * awc])
                        i = t * anwc + c
                        nc.vector.tensor_reduce(part[:, i:i + 1], wt_[:],
                                                axis=ax_x, op=mx,
                                                apply_absolute_value=True)
            with tc.tile_pool(name="wprep", bufs=2) as wp:
                cmax = misc[:, 258:259]
                nc.vector.tensor_reduce(cmax, part[:], axis=ax_x, op=mx,
                                        apply_absolute_value=True)
                cmax_all = misc[:, 262:263]
                from concourse.bass import bass_isa
                nc.gpsimd.partition_all_reduce(cmax_all, cmax, 128,
                                               bass_isa.ReduceOp.max)
                nc.sync.dma_start(cc_in, cmax_all[0:1, 0:1])
                if n_cores > 1:
                    nc.gpsimd.collective_compute(
                        "AllReduce", mx,
                        replica_groups=[list(range(n_cores))],
                        ins=[cc_in], outs=[cc_out])
                else:
                    nc.sync.dma_start(cc_out, cc_in)
                nc.sync.dma_start(amax, cc_out)
                nc.vector.tensor_scalar(scale_s, amax, R7, None, mult)
                nc.vector.tensor_scalar(scale_s, scale_s, EPS, None, mx)
                pb = psprep.tile([128, 1], f32, tag="brd", name="pb")
                nc.tensor.matmul(pb[:], ones, scale_s, start=True, stop=True)
                nc.scalar.copy(scale_col, pb[:])
                nc.vector.reciprocal(inv_col, scale_col)

                # ---- quantize -> q_dram panels ----
                for c in range(nwc):
                    for t in range(wrows):
                        wt_ = wp.tile([128, wc], f32, tag="wload")
                        nc.sync.dma_start(wt_[:], wd[t * 128:(t + 1) * 128,
                                                     c * wc:(c + 1) * wc])
                        nc.vector.tensor_scalar(wt_[:], wt_[:],
                                                inv_col, FP_MAGIC, mult, add)
                        qt = wp.tile([128, wc], bf16, tag="wq")
                        nc.vector.tensor_scalar(qt[:], wt_[:], FP_MAGIC,
                                                None, sub)
                        for s in range(wc // tch):
                            pc = (c * wc) // tch + s
                            nc.sync.dma_start(
                                qds[pc][t * 128:(t + 1) * 128, :],
                                qt[:, s * tch:(s + 1) * tch])

            # ---- transpose panels -> resident fp8 (head) / bf16 (tail) ----
            with tc.tile_pool(name="wtr", bufs=2) as wtrp:
                for c in range(nch):
                    t0 = c * tpc
                    if t0 < kta:  # full-A panel (alignment asserted above)
                        tr = wtrp.tile([128, tpc, out_sh], bf16, tag="wtr")
                        nc.sync.dma_start_transpose(tr[:], qds[c][:, :])
                        nc.scalar.copy(wt8[:, t0:t0 + tpc, :], tr[:])
                    else:
                        nc.sync.dma_start_transpose(
                            wtb[:, t0 - kta:t0 - kta + tpc, :], qds[c][:, :])

        # ---- main loop ----
        with tc.tile_pool(name="psum", bufs=2, space="PSUM") as psp:
            for b in range(nb):
                xt = xtp.tile([128, kt, 128], bf16, tag="xt")
                x8 = (x8p.tile([128, kta, 128], f8e4, tag="x8") if kta
                      else None)
                for c2 in range(nxc):
                    xf = xfp.tile([128, xc], f32, tag="xf")
                    nc.sync.dma_start(xf[:], xd[b * 128:(b + 1) * 128,
                                                c2 * xc:(c2 + 1) * xc])
                    xbt = xbp.tile([128, xc], bf16, tag="xb")
                    nc.scalar.copy(xbt[:], xf[:])
                    nc.sync.dma_start_transpose(
                        xt[:, c2 * xkc:(c2 + 1) * xkc, :], xbt[:])
                    lo, hi = c2 * xkc, min((c2 + 1) * xkc, kta)
                    if hi > lo:
                        nc.vector.tensor_scalar(
                            x8[:, lo:hi, :], xt[:, lo:hi, :], 1.0, None, mult)
                psums = [psp.tile([128, ot], f32, tag=f"mm{j}", name=f"ps{j}")
                         for j in range(not_)]
                for p in range(kta // 2):
                    for j in range(not_):
                        nc.tensor.matmul(
                            psums[j][:], x8[:, 2 * p:2 * p + 2, :],
                            wt8[:, 2 * p:2 * p + 2, j * ot:(j + 1) * ot],
                            start=(p == 0),
                            stop=(KTB == 0 and p == kta // 2 - 1),
                            perf_mode=drow)
                for t in range(KTB):
                    for j in range(not_):
                        nc.tensor.matmul(
                            psums[j][:], xt[:, kta + t, :],
                            wtb[:, t, j * ot:(j + 1) * ot],
                            start=(kta == 0 and t == 0),
                            stop=(t == KTB - 1))
                for j in range(not_):
                    ob = obp.tile([128, ot], f32, tag="ob")
                    co = j * ot
                    nc.vector.scalar_tensor_tensor(
                        ob[:], psums[j][:], scale_col, bias_rep[:, co:co + ot],
                        mult, add)
                    nc.sync.dma_start(outd[b * 128:(b + 1) * 128,
                                           co:co + ot], ob[:])


def build_nc(n=N_FULL, in_=IN_FULL, out_sh=OUT_FULL // CORES, n_cores=CORES,
             rep=1):
    nc = bacc.Bacc("TRN2", target_bir_lowering=False, debug=False,
                   enable_asserts=False, num_devices=n_cores)
    with tile.TileContext(nc) as tc:
        io = declare_io(nc, n, in_, out_sh)
        for _ in range(rep):
            emit_program(tc, n, in_, out_sh, n_cores, io=io)
    nc.compile()
    return nc


_NC_CACHE = {}


def _get_nc():
    key = (N_FULL, IN_FULL, OUT_FULL, CORES)
    if key not in _NC_CACHE:
        _NC_CACHE[key] = build_nc()
    return _NC_CACHE[key]


def kernel(x: np.ndarray, weight: np.ndarray, bias: np.ndarray) -> np.ndarray:
    assert x.shape == (N_FULL, IN_FULL)
    assert weight.shape == (OUT_FULL, IN_FULL)
    assert bias.shape == (OUT_FULL,)
    x = np.ascontiguousarray(x, dtype=np.float32)
    weight = np.ascontiguousarray(weight, dtype=np.float32)
    bias = np.ascontiguousarray(bias, dtype=np.float32)

    osh = OUT_FULL // CORES
    nc = _get_nc()
    in_maps = [
        {"x": x,
         "weight": weight[i * osh:(i + 1) * osh],
         "bias": bias[i * osh:(i + 1) * osh]}
        for i in range(CORES)
    ]
    res = run_bass_kernel_spmd(nc, in_maps, list(range(CORES))).results
    return np.concatenate([res[i]["out"] for i in range(CORES)], axis=1)


# revision 4
# speedup vs baseline: 13.3669x; 13.3669x over previous
"""Trainium2 Bass kernel for nn_CANNLinear (4-bit per-tensor symmetric weight
quantization + dense linear), column-parallel over 8 NeuronCores.

Computation (matches the reference exactly at the quantization step):
    scale  = max(max(|W|) * fl(1/7), 1e-8)        (global over full W, AllReduce max)
    q      = round(W * fl(1/scale))               (RNE round via +/-1.5*2^23)
    out    = x @ (q * scale)^T + bias

Matmul runs in mixed precision to hit the fp8 double-pump (157 TF/s) while
staying inside the 2e-2 rel-err gate:
  - KTA k-tiles: x cast f32->bf16->e4m3, q cast to e4m3 (exact: q in [-8,7]),
    DoubleRow perf mode consumes k-tile PAIRS at 2x bf16 rate.
  - KTB tail k-tiles: x bf16, q bf16 (exact), normal matmul.
Host-side exact simulation on the benchmark data (sim_err.py): KTB=20 ->
rel err 0.0167; pure fp8 would be 0.0283 (fail), pure bf16 0.0017.

Sharding: W/bias split along OUT across 8 cores (column parallel), x replicated,
per-core output [N, OUT/8] concatenated on the host along axis 1.

Per-core program:
  1. bias broadcast to [128, OUT_SH] via PE outer product (off critical path)
  2. absmax: stream W, DVE abs-max reduce -> AllReduce(max) -> scale, 1/scale
  3. quantize: re-stream W, q_bf16 = (w*inv + M) - M -> q_dram (8 panels of
     512 cols), xbar DMA-transpose panels -> fp8 cast (first KTA tiles) or
     direct bf16 residency (tail KTB tiles); low-k matmuls start while later
     panels still stream
  4. main: per 128-row block: load x halves, ACT cast bf16, xbar-transpose,
     DVE cast tiles 0..KTA-1 to e4m3; 4 PSUM banks accumulate KTA/2 DoubleRow
     + KTB bf16 matmuls; epilogue ACT copy*scale + DVE bias add -> DMA out
"""

import numpy as np

import concourse.bass as bass
import concourse.mybir as mybir
import concourse.tile as tile
from concourse import bacc
from concourse.bass_utils import run_bass_kernel_spmd

f32 = mybir.dt.float32
bf16 = mybir.dt.bfloat16
f8e4 = mybir.dt.float8e4
FP_MAGIC = 12582912.0  # 1.5 * 2**23: v + FP_MAGIC - FP_MAGIC == round-half-even(v)
QMAX = 7.0
R7 = float(np.float32(1.0) / np.float32(7.0))  # fl(1/7)
EPS = 1e-8

N_FULL, IN_FULL, OUT_FULL = 8192, 4096, 16384
CORES = 8
KTB = 20  # bf16 tail k-tiles of kt=32; KTA = kt - KTB fp8 (must be even)


def declare_io(nc, n, in_, out_sh):
    xd = nc.dram_tensor("x", [n, in_], f32, kind="ExternalInput").ap()
    wd = nc.dram_tensor("weight", [out_sh, in_], f32, kind="ExternalInput").ap()
    bd = nc.dram_tensor("bias", [out_sh], f32, kind="ExternalInput").ap()
    outd = nc.dram_tensor("out", [n, out_sh], f32, kind="ExternalOutput").ap()
    return xd, wd, bd, outd


_REP_ID = [0]


def emit_program(tc, n, in_, out_sh, n_cores, io=None):
    nc = tc.nc
    if io is None:
        io = declare_io(nc, n, in_, out_sh)
    xd, wd, bd, outd = io
    rid = _REP_ID[0]
    _REP_ID[0] += 1
    add = mybir.AluOpType.add
    sub = mybir.AluOpType.subtract
    mult = mybir.AluOpType.mult
    mx = mybir.AluOpType.max
    ax_x = mybir.AxisListType.X
    drow = mybir.MatmulPerfMode.DoubleRow

    kt = in_ // 128          # contraction tiles (32)
    kta = kt - KTB           # fp8 double-row tiles
    assert kta % 2 == 0 and kta >= 0
    nb = n // 128            # row blocks
    ot = min(512, out_sh)    # psum tile free dim
    not_ = out_sh // ot      # psum groups per row block (<=4 for 8 banks)
    assert not_ <= 4
    wrows = out_sh // 128
    wc = min(in_, 2048)      # quant load chunk columns
    nwc = in_ // wc
    tch = 512                # transpose panel columns (4 k-tiles)
    nch = in_ // tch
    tpc = tch // 128         # k-tiles per panel (4)
    assert kta % tpc == 0, "fp8/bf16 boundary must align to transpose panels"
    xc = min(in_, 2048)      # x load chunk columns
    nxc = in_ // xc
    xkc = xc // 128

    qds = [nc.dram_tensor(f"q_dram{rid}_{c}", [out_sh, tch], bf16).ap()
           for c in range(nch)]
    cc_in = nc.dram_tensor(f"cc_in{rid}", [1], f32).ap()
    cc_out = nc.dram_tensor(f"cc_out{rid}", [1], f32, addr_space="Shared").ap()

    from contextlib import ExitStack

    with ExitStack() as ctx:
        const = ctx.enter_context(tc.tile_pool(name="const", bufs=1))
        xfp = ctx.enter_context(tc.tile_pool(name="xf", bufs=2))
        xbp = ctx.enter_context(tc.tile_pool(name="xb", bufs=2))
        xtp = ctx.enter_context(tc.tile_pool(name="xt", bufs=2))
        x8p = ctx.enter_context(tc.tile_pool(name="x8", bufs=2))
        obp = ctx.enter_context(tc.tile_pool(name="ob", bufs=2))
        wtp = ctx.enter_context(tc.tile_pool(name="wt", bufs=1))

        # one padded slot holds all tiny scalar tiles
        nwt = wrows * (in_ // min(in_, 1024))
        misc = const.tile([128, 272 + nwt], f32, tag="misc")
        ones = misc[0:1, 0:128]
        scale_col = misc[:, 256:257]
        inv_col = misc[:, 257:258]
        amax = misc[0:1, 259:260]
        scale_s = misc[0:1, 260:261]
        part = misc[:, 272:272 + nwt]
        bias_rep = const.tile([128, out_sh], f32, tag="bias_rep")
        wt8 = (wtp.tile([128, kta, out_sh], f8e4, tag="wt8", name="wt8")
               if kta else None)
        wtb = (wtp.tile([128, KTB, out_sh], bf16, tag="wtb", name="wtb")
               if KTB else None)

        nc.vector.memset(ones, 1.0)

        # ---- bias broadcast (independent of everything else) ----
        with tc.tile_pool(name="psprep", bufs=2, space="PSUM") as psprep:
            nc.sync.dma_start(bias_rep[0:1, :], bd)
            for j in range(out_sh // ot):
                pbias = psprep.tile([128, ot], f32, tag="brd", name="pbias")
                nc.tensor.matmul(pbias[:], ones,
                                 bias_rep[0:1, j * ot:(j + 1) * ot],
                                 start=True, stop=True)
                nc.scalar.copy(bias_rep[:, j * ot:(j + 1) * ot], pbias[:])

            # ---- absmax -> scale (own deep pool: DMA-rate streaming) ----
            awc = min(in_, 1024)
            anwc = in_ // awc
            with tc.tile_pool(name="wabs", bufs=6) as wabs:
                for t in range(wrows):
                    for c in range(anwc):
                        wt_ = wabs.tile([128, awc], f32, tag="aload")
                        nc.sync.dma_start(wt_[:], wd[t * 128:(t + 1) * 128,
                                                     c * awc:(c + 1) * awc])
                        i = t * anwc + c
                        nc.vector.tensor_reduce(part[:, i:i + 1], wt_[:],
                                                axis=ax_x, op=mx,
                                                apply_absolute_value=True)
            with tc.tile_pool(name="wprep", bufs=2) as wp:
                cmax = misc[:, 258:259]
                nc.vector.tensor_reduce(cmax, part[:], axis=ax_x, op=mx,
                                        apply_absolute_value=True)
                cmax_all = misc[:, 262:263]
                from concourse.bass import bass_isa
                nc.gpsimd.partition_all_reduce(cmax_all, cmax, 128,
                                               bass_isa.ReduceOp.max)
                nc.sync.dma_start(cc_in, cmax_all[0:1, 0:1])
                if n_cores > 1:
                    nc.gpsimd.collective_compute(
                        "AllReduce", mx,
                        replica_groups=[list(range(n_cores))],
                        ins=[cc_in], outs=[cc_out])
                else:
                    nc.sync.dma_start(cc_out, cc_in)
                nc.sync.dma_start(amax, cc_out)
                nc.vector.tensor_scalar(scale_s, amax, R7, None, mult)
                nc.vector.tensor_scalar(scale_s, scale_s, EPS, None, mx)
                pb = psprep.tile([128, 1], f32, tag="brd", name="pb")
                nc.tensor.matmul(pb[:], ones, scale_s, start=True, stop=True)
                nc.scalar.copy(scale_col, pb[:])
                nc.vector.reciprocal(inv_col, scale_col)

                # ---- quantize -> q_dram panels ----
                for c in range(nwc):
                    for t in range(wrows):
                        wt_ = wp.tile([128, wc], f32, tag="wload")
                        nc.sync.dma_start(wt_[:], wd[t * 128:(t + 1) * 128,
                                                     c * wc:(c + 1) * wc])
                        nc.vector.tensor_scalar(wt_[:], wt_[:],
                                                inv_col, FP_MAGIC, mult, add)
                        qt = wp.tile([128, wc], bf16, tag="wq")
                        nc.vector.tensor_scalar(qt[:], wt_[:], FP_MAGIC,
                                                None, sub)
                        for s in range(wc // tch):
                            pc = (c * wc) // tch + s
                            nc.sync.dma_start(
                                qds[pc][t * 128:(t + 1) * 128, :],
                                qt[:, s * tch:(s + 1) * tch])

            # ---- transpose panels -> resident fp8 (head) / bf16 (tail) ----
            with tc.tile_pool(name="wtr", bufs=2) as wtrp:
                for c in range(nch):
                    t0 = c * tpc
                    if t0 < kta:  # full-A panel (alignment asserted above)
                        tr = wtrp.tile([128, tpc, out_sh], bf16, tag="wtr")
                        nc.sync.dma_start_transpose(tr[:], qds[c][:, :])
                        nc.scalar.copy(wt8[:, t0:t0 + tpc, :], tr[:])
                    else:
                        nc.sync.dma_start_transpose(
                            wtb[:, t0 - kta:t0 - kta + tpc, :], qds[c][:, :])

        # ---- main loop ----
        with tc.tile_pool(name="psum", bufs=2, space="PSUM") as psp:
            for b in range(nb):
                xt = xtp.tile([128, kt, 128], bf16, tag="xt")
                x8 = (x8p.tile([128, kta, 128], f8e4, tag="x8", name="x8")
                      if kta else None)
                for c2 in range(nxc):
                    xf = xfp.tile([128, xc], f32, tag="xf")
                    nc.sync.dma_start(xf[:], xd[b * 128:(b + 1) * 128,
                                                c2 * xc:(c2 + 1) * xc])
                    xbt = xbp.tile([128, xc], bf16, tag="xb")
                    nc.scalar.copy(xbt[:], xf[:])
                    nc.sync.dma_start_transpose(
                        xt[:, c2 * xkc:(c2 + 1) * xkc, :], xbt[:])
                    lo, hi = c2 * xkc, min((c2 + 1) * xkc, kta)
                    if hi > lo:
                        nc.vector.tensor_scalar(
                            x8[:, lo:hi, :], xt[:, lo:hi, :], 1.0, None, mult)
                psums = [psp.tile([128, ot], f32, tag=f"mm{j}", name=f"ps{j}")
                         for j in range(not_)]
                for p in range(kta // 2):
                    for j in range(not_):
                        nc.tensor.matmul(
                            psums[j][:], x8[:, 2 * p:2 * p + 2, :],
                            wt8[:, 2 * p:2 * p + 2, j * ot:(j + 1) * ot],
                            start=(p == 0),
                            stop=(KTB == 0 and p == kta // 2 - 1),
                            perf_mode=drow)
                for t in range(KTB):
                    for j in range(not_):
                        nc.tensor.matmul(
                            psums[j][:], xt[:, kta + t, :],
                            wtb[:, t, j * ot:(j + 1) * ot],
                            start=(kta == 0 and t == 0),
                            stop=(t == KTB - 1))
                for j in range(not_):
                    ob = obp.tile([128, ot], f32, tag="ob")
                    co = j * ot
                    nc.vector.scalar_tensor_tensor(
                        ob[:], psums[j][:], scale_col, bias_rep[:, co:co + ot],
                        mult, add)
                    nc.sync.dma_start(outd[b * 128:(b + 1) * 128,
                                           co:co + ot], ob[:])


def build_nc(n=N_FULL, in_=IN_FULL, out_sh=OUT_FULL // CORES, n_cores=CORES,
             rep=1):
    nc = bacc.Bacc("TRN2", target_bir_lowering=False, debug=False,
                   enable_asserts=False, num_devices=n_cores)
    with tile.TileContext(nc) as tc:
        io = declare_io(nc, n, in_, out_sh)
        for _ in range(rep):
            emit_program(tc, n, in_, out_sh, n_cores, io=io)
    nc.compile()
    return nc


_NC_CACHE = {}


def _get_nc():
    key = (N_FULL, IN_FULL, OUT_FULL, CORES)
    if key not in _NC_CACHE:
        _NC_CACHE[key] = build_nc()
    return _NC_CACHE[key]


def kernel(x: np.ndarray, weight: np.ndarray, bias: np.ndarray) -> np.ndarray:
    assert x.shape == (N_FULL, IN_FULL)
    assert weight.shape == (OUT_FULL, IN_FULL)
    assert bias.shape == (OUT_FULL,)
    x = np.ascontiguousarray(x, dtype=np.float32)
    weight = np.ascontiguousarray(weight, dtype=np.float32)
    bias = np.ascontiguousarray(bias, dtype=np.float32)

    osh = OUT_FULL // CORES
    nc = _get_nc()
    in_maps = [
        {"x": x,
         "weight": weight[i * osh:(i + 1) * osh],
         "bias": bias[i * osh:(i + 1) * osh]}
        for i in range(CORES)
    ]
    res = run_bass_kernel_spmd(nc, in_maps, list(range(CORES))).results
    return np.concatenate([res[i]["out"] for i in range(CORES)], axis=1)


# revision 8
# speedup vs baseline: 132.5498x; 9.9163x over previous
"""Trainium2 Bass kernel for nn_CANNLinear (4-bit per-tensor symmetric weight
quantization + dense linear), column-parallel over 8 NeuronCores.

Computation (matches the reference exactly at the quantization step):
    scale  = max(max(|W|) * fl(1/7), 1e-8)        (global over full W, AllReduce max)
    q      = round(W * fl(1/scale))               (RNE round via +/-1.5*2^23)
    out    = x @ (q * scale)^T + bias

Matmul runs in mixed precision to hit the fp8 double-pump (157 TF/s) while
staying inside the 2e-2 rel-err gate:
  - KTA k-tiles: x cast f32->bf16->e4m3, q cast to e4m3 (exact: q in [-8,7]),
    DoubleRow perf mode consumes k-tile PAIRS at 2x bf16 rate.
  - KTB tail k-tiles: x bf16, q bf16 (exact), normal matmul.
Host-side exact simulation on the benchmark data (sim_err.py): KTB=20 ->
rel err 0.0167; pure fp8 would be 0.0283 (fail), pure bf16 0.0017.

Sharding: W/bias split along OUT across 8 cores (column parallel), x replicated,
per-core output [N, OUT/8] concatenated on the host along axis 1.

Per-core program:
  1. bias broadcast to [128, OUT_SH] via PE outer product (off critical path)
  2. absmax: stream W, DVE abs-max reduce -> AllReduce(max) -> scale, 1/scale
  3. quantize: re-stream W, q_bf16 = (w*inv + M) - M -> q_dram (8 panels of
     512 cols), xbar DMA-transpose panels -> fp8 cast (first KTA tiles) or
     direct bf16 residency (tail KTB tiles); low-k matmuls start while later
     panels still stream
  4. main: per 128-row block: load x halves, ACT cast bf16, xbar-transpose,
     DVE cast tiles 0..KTA-1 to e4m3; 4 PSUM banks accumulate KTA/2 DoubleRow
     + KTB bf16 matmuls; epilogue ACT copy*scale + DVE bias add -> DMA out
"""

import numpy as np

import concourse.bass as bass
import concourse.mybir as mybir
import concourse.tile as tile
from concourse import bacc
from concourse.bass_utils import run_bass_kernel_spmd

f32 = mybir.dt.float32
bf16 = mybir.dt.bfloat16
f8e4 = mybir.dt.float8e4
FP_MAGIC = 12582912.0  # 1.5 * 2**23: v + FP_MAGIC - FP_MAGIC == round-half-even(v)
QMAX = 7.0
R7 = float(np.float32(1.0) / np.float32(7.0))  # fl(1/7)
EPS = 1e-8

N_FULL, IN_FULL, OUT_FULL = 8192, 4096, 16384
CORES = 8
KTB = 20  # bf16 tail k-tiles of kt=32; KTA = kt - KTB fp8 (must be even)


def declare_io(nc, n, in_, out_sh):
    xd = nc.dram_tensor("x", [n, in_], f32, kind="ExternalInput").ap()
    wd = nc.dram_tensor("weight", [out_sh, in_], f32, kind="ExternalInput").ap()
    bd = nc.dram_tensor("bias", [out_sh], f32, kind="ExternalInput").ap()
    outd = nc.dram_tensor("out", [n, out_sh], f32, kind="ExternalOutput").ap()
    return xd, wd, bd, outd


_REP_ID = [0]


def emit_program(tc, n, in_, out_sh, n_cores, io=None):
    nc = tc.nc
    if io is None:
        io = declare_io(nc, n, in_, out_sh)
    xd, wd, bd, outd = io
    rid = _REP_ID[0]
    _REP_ID[0] += 1
    add = mybir.AluOpType.add
    sub = mybir.AluOpType.subtract
    mult = mybir.AluOpType.mult
    mx = mybir.AluOpType.max
    ax_x = mybir.AxisListType.X
    drow = mybir.MatmulPerfMode.DoubleRow

    kt = in_ // 128          # contraction tiles (32)
    kta = kt - KTB           # fp8 double-row tiles
    assert kta % 2 == 0 and kta >= 0
    nb = n // 128            # row blocks
    ot = min(512, out_sh)    # psum tile free dim
    not_ = out_sh // ot      # psum groups per row block (<=4 for 8 banks)
    assert not_ <= 4
    wrows = out_sh // 128
    wc = min(in_, 1024)      # quant load chunk columns
    nwc = in_ // wc
    tch = 512                # transpose panel columns (4 k-tiles)
    nch = in_ // tch
    tpc = tch // 128         # k-tiles per panel (4)
    assert kta % tpc == 0, "fp8/bf16 boundary must align to transpose panels"
    xc = min(in_, 2048)      # x load chunk columns
    nxc = in_ // xc
    xkc = xc // 128

    qds = [nc.dram_tensor(f"q_dram{rid}_{c}", [out_sh, tch], bf16).ap()
           for c in range(nch)]
    cc_in = nc.dram_tensor(f"cc_in{rid}", [1], f32).ap()
    cc_out = nc.dram_tensor(f"cc_out{rid}", [1], f32, addr_space="Shared").ap()

    from contextlib import ExitStack

    with ExitStack() as ctx:
        const = ctx.enter_context(tc.tile_pool(name="const", bufs=1))
        xfp = ctx.enter_context(tc.tile_pool(name="xf", bufs=2))
        xbp = ctx.enter_context(tc.tile_pool(name="xb", bufs=2))
        xtp = ctx.enter_context(tc.tile_pool(name="xt", bufs=2))
        x8p = ctx.enter_context(tc.tile_pool(name="x8", bufs=2))
        obp = ctx.enter_context(tc.tile_pool(name="ob", bufs=2))
        wtp = ctx.enter_context(tc.tile_pool(name="wt", bufs=1))

        # one padded slot holds all tiny scalar tiles
        nwt = wrows * (in_ // min(in_, 1024))
        misc = const.tile([128, 272 + nwt], f32, tag="misc")
        ones = misc[0:1, 0:128]
        scale_col = misc[:, 256:257]
        inv_col = misc[:, 257:258]
        amax = misc[0:1, 259:260]
        scale_s = misc[0:1, 260:261]
        part = misc[:, 272:272 + nwt]
        bias_rep = const.tile([128, out_sh], f32, tag="bias_rep")
        wt8 = (wtp.tile([128, kta, out_sh], f8e4, tag="wt8", name="wt8")
               if kta else None)
        wtb = (wtp.tile([128, KTB, out_sh], bf16, tag="wtb", name="wtb")
               if KTB else None)

        nc.vector.memset(ones, 1.0)

        # ---- bias broadcast (independent of everything else) ----
        with tc.tile_pool(name="psprep", bufs=2, space="PSUM") as psprep:
            nc.sync.dma_start(bias_rep[0:1, :], bd)
            for j in range(out_sh // ot):
                pbias = psprep.tile([128, ot], f32, tag="brd", name="pbias")
                nc.tensor.matmul(pbias[:], ones,
                                 bias_rep[0:1, j * ot:(j + 1) * ot],
                                 start=True, stop=True)
                nc.scalar.copy(bias_rep[:, j * ot:(j + 1) * ot], pbias[:])

            # ---- absmax -> scale (own deep pool: DMA-rate streaming) ----
            awc = min(in_, 1024)
            anwc = in_ // awc
            with tc.tile_pool(name="wabs", bufs=4) as wabs:
                for t in range(wrows):
                    for c in range(anwc):
                        wt_ = wabs.tile([128, awc], f32, tag="aload")
                        nc.sync.dma_start(wt_[:], wd[t * 128:(t + 1) * 128,
                                                     c * awc:(c + 1) * awc])
                        i = t * anwc + c
                        nc.vector.tensor_reduce(part[:, i:i + 1], wt_[:],
                                                axis=ax_x, op=mx,
                                                apply_absolute_value=True)
            with tc.tile_pool(name="wprep", bufs=2) as wp, \
                 tc.tile_pool(name="wload", bufs=4) as wlp, \
                 tc.tile_pool(name="wtr", bufs=1) as wtrp:
                cmax = misc[:, 258:259]
                nc.vector.tensor_reduce(cmax, part[:], axis=ax_x, op=mx,
                                        apply_absolute_value=True)
                cmax_all = misc[:, 262:263]
                from concourse.bass import bass_isa
                nc.gpsimd.partition_all_reduce(cmax_all, cmax, 128,
                                               bass_isa.ReduceOp.max)
                nc.sync.dma_start(cc_in, cmax_all[0:1, 0:1])
                if n_cores > 1:
                    nc.gpsimd.collective_compute(
                        "AllReduce", mx,
                        replica_groups=[list(range(n_cores))],
                        ins=[cc_in], outs=[cc_out])
                else:
                    nc.sync.dma_start(cc_out, cc_in)
                nc.sync.dma_start(amax, cc_out)
                nc.vector.tensor_scalar(scale_s, amax, R7, None, mult)
                nc.vector.tensor_scalar(scale_s, scale_s, EPS, None, mx)
                pb = psprep.tile([128, 1], f32, tag="brd", name="pb")
                nc.tensor.matmul(pb[:], ones, scale_s, start=True, stop=True)
                nc.scalar.copy(scale_col, pb[:])
                nc.vector.reciprocal(inv_col, scale_col)

                # ---- quantize -> q_dram panels; transpose each chunk's
                # panels as soon as they complete. W loads have no dep on
                # scale, so they prefetch during absmax/AllReduce; DVE does
                # mult+add, ACT does the -MAGIC sub (pipelined engines). ----
                copy_f = mybir.ActivationFunctionType.Copy
                for c in range(nwc):
                    for t in range(wrows):
                        wl = wlp.tile([128, wc], f32, tag="wload")
                        nc.sync.dma_start(wl[:], wd[t * 128:(t + 1) * 128,
                                                    c * wc:(c + 1) * wc])
                        tf = wp.tile([128, wc], f32, tag="wtmp")
                        nc.vector.tensor_scalar(tf[:], wl[:],
                                                inv_col, FP_MAGIC, mult, add)
                        qt = wp.tile([128, wc], bf16, tag="wq")
                        nc.scalar.activation(qt[:], tf[:], copy_f,
                                             bias=-FP_MAGIC, scale=1.0)
                        for s in range(wc // tch):
                            pc = (c * wc) // tch + s
                            nc.sync.dma_start(
                                qds[pc][t * 128:(t + 1) * 128, :],
                                qt[:, s * tch:(s + 1) * tch])
                    for s in range(wc // tch):
                        pc = (c * wc) // tch + s
                        t0 = pc * tpc
                        if t0 < kta:  # full-A panel (alignment asserted)
                            tr = wtrp.tile([128, tpc, out_sh], bf16,
                                           tag="wtr")
                            nc.sync.dma_start_transpose(tr[:], qds[pc][:, :])
                            nc.vector.tensor_scalar(
                                wt8[:, t0:t0 + tpc, :], tr[:], 1.0, None,
                                mult)
                        else:
                            nc.sync.dma_start_transpose(
                                wtb[:, t0 - kta:t0 - kta + tpc, :],
                                qds[pc][:, :])

        # ---- main loop ----
        with tc.tile_pool(name="psum", bufs=2, space="PSUM") as psp:
            for b in range(nb):
                xt = xtp.tile([128, kt, 128], bf16, tag="xt")
                x8 = (x8p.tile([128, kta, 128], f8e4, tag="x8", name="x8")
                      if kta else None)
                for c2 in range(nxc):
                    xf = xfp.tile([128, xc], f32, tag="xf")
                    nc.sync.dma_start(xf[:], xd[b * 128:(b + 1) * 128,
                                                c2 * xc:(c2 + 1) * xc])
                    xbt = xbp.tile([128, xc], bf16, tag="xb")
                    nc.scalar.copy(xbt[:], xf[:])
                    nc.sync.dma_start_transpose(
                        xt[:, c2 * xkc:(c2 + 1) * xkc, :], xbt[:])
                    lo, hi = c2 * xkc, min((c2 + 1) * xkc, kta)
                    if hi > lo:
                        nc.vector.tensor_scalar(
                            x8[:, lo:hi, :], xt[:, lo:hi, :], 1.0, None, mult)
                psums = [psp.tile([128, ot], f32, tag=f"mm{j}", name=f"ps{j}")
                         for j in range(not_)]
                for p in range(kta // 2):
                    for j in range(not_):
                        nc.tensor.matmul(
                            psums[j][:], x8[:, 2 * p:2 * p + 2, :],
                            wt8[:, 2 * p:2 * p + 2, j * ot:(j + 1) * ot],
                            start=(p == 0),
                            stop=(KTB == 0 and p == kta // 2 - 1),
                            perf_mode=drow)
                for t in range(KTB):
                    for j in range(not_):
                        nc.tensor.matmul(
                            psums[j][:], xt[:, kta + t, :],
                            wtb[:, t, j * ot:(j + 1) * ot],
                            start=(kta == 0 and t == 0),
                            stop=(t == KTB - 1))
                for j in range(not_):
                    ob = obp.tile([128, ot], f32, tag="ob")
                    co = j * ot
                    nc.vector.scalar_tensor_tensor(
                        ob[:], psums[j][:], scale_col, bias_rep[:, co:co + ot],
                        mult, add)
                    nc.sync.dma_start(outd[b * 128:(b + 1) * 128,
                                           co:co + ot], ob[:])


def build_nc(n=N_FULL, in_=IN_FULL, out_sh=OUT_FULL // CORES, n_cores=CORES,
             rep=1):
    nc = bacc.Bacc("TRN2", target_bir_lowering=False, debug=False,
                   enable_asserts=False, num_devices=n_cores)
    with tile.TileContext(nc) as tc:
        io = declare_io(nc, n, in_, out_sh)
        for _ in range(rep):
            emit_program(tc, n, in_, out_sh, n_cores, io=io)
    nc.compile()
    return nc


_NC_CACHE = {}


def _get_nc():
    key = (N_FULL, IN_FULL, OUT_FULL, CORES)
    if key not in _NC_CACHE:
        _NC_CACHE[key] = build_nc()
    return _NC_CACHE[key]


def kernel(x: np.ndarray, weight: np.ndarray, bias: np.ndarray) -> np.ndarray:
    assert x.shape == (N_FULL, IN_FULL)
    assert weight.shape == (OUT_FULL, IN_FULL)
    assert bias.shape == (OUT_FULL,)
    x = np.ascontiguousarray(x, dtype=np.float32)
    weight = np.ascontiguousarray(weight, dtype=np.float32)
    bias = np.ascontiguousarray(bias, dtype=np.float32)

    osh = OUT_FULL // CORES
    nc = _get_nc()
    in_maps = [
        {"x": x,
         "weight": weight[i * osh:(i + 1) * osh],
         "bias": bias[i * osh:(i + 1) * osh]}
        for i in range(CORES)
    ]
    res = run_bass_kernel_spmd(nc, in_maps, list(range(CORES))).results
    return np.concatenate([res[i]["out"] for i in range(CORES)], axis=1)
